# revision 14
# baseline (speedup 1.0000x reference)
"""DiffWire GNN forward on 8 Trainium2 NeuronCores — data-parallel over B=16 graphs.

Self-contained: builds a Bass/Tile program (2 graphs per core), shards inputs,
runs SPMD on cores 0-7, gathers logits + per-graph loss terms on host.

Math restructurings (validated vs the reference in fp32/bf16 numpy sims):
 - adj_gap = A*(1-(f_u-f_v)^2) products expanded into matmuls vs raw A
   (A@[x, f*x, f^2*x, ...]) -> no N^2 elementwise work in the GAP branch.
 - CT pairwise distances d2 = |s_u-s_v|^2 as one PE matmul with augmented
   factors [s|sq|1] @ [-2s|1|sq]^T; DVE clamp (max(.,0)+1e-5) + ACT sqrt.
 - ortho loss ||ss/n - I/sqrt(k)||_F = sqrt(2 - (2/sqrt(k))*tr(ss)/n).
 - mincut ratios are scale-invariant -> 1/vol applied once to conv_ct.
 - bf16 matmuls, fp32 PSUM accumulate; hi/lo (pseudo-fp32) on the
   logits-critical path, merged into single wide-rhs matmuls where possible.
"""
import contextlib
import os
DEBUG = bool(int(os.environ.get("KDBG", "0")))

import numpy as np
import ml_dtypes

import concourse.bass as bass
import concourse.mybir as mybir
from concourse import bacc, bass_isa
from concourse.tile import TileContext

BF = mybir.dt.bfloat16
F32 = mybir.dt.float32
AF = mybir.ActivationFunctionType
ALU = mybir.AluOpType
AX = mybir.AxisListType
RED = bass_isa.ReduceOp

N = 1024
NB = 8            # node blocks of 128
GPC = 2           # graphs per core
NCORES = 8
NOUT = 10
EPS = 1e-15
SQRT_BIAS = 1e-5
VW = 264          # V block width (hi 0:132 | lo 132:264)
DPW = 68          # dpack cols: s 0:32|sq 32|1 33|-2s 34:66|1 66|sq 67
XC = 66           # XEZ cols: x_emb 0:32 | z_rw 32:34 | z_ct 34:66


def _bf(a):
    return np.asarray(a, np.float32).astype(ml_dtypes.bfloat16)


def _hilo(a):
    h = _bf(a)
    l = _bf(np.asarray(a, np.float32) - h.astype(np.float32))
    return h, l


WSPECS = [
    ("wstk1_h", [128, 66], BF), ("wstk1_l", [128, 66], BF), ("bstk1", [1, 66], F32),
    ("grel48_h", [32, 48], BF), ("grel48_l", [32, 48], BF), ("bg48", [1, 48], F32),
    ("groot48_h", [32, 48], BF), ("groot48_l", [32, 48], BF),
    ("croot48_h", [32, 48], BF), ("croot48_l", [32, 48], BF), ("bc48", [1, 48], F32),
    ("ct48", [32, 48], BF),
    ("w2grel_h", [32, 32], BF), ("w2grel_l", [32, 32], BF),
    ("w2groot_h", [32, 32], BF), ("w2groot_l", [32, 32], BF),
    ("b2gcol", [32, 1], F32),
    ("wcatg_h", [32, 32], BF), ("wcatg_l", [32, 32], BF),
    ("wcatc_h", [32, 32], BF), ("wcatc_l", [32, 32], BF), ("bcat", [1, 32], F32),
    ("w2_h", [32, 32], BF), ("w2_l", [32, 32], BF), ("b2col", [32, 1], F32),
    ("w3_h", [32, 10], BF), ("w3_l", [32, 10], BF), ("b3col", [10, 1], F32),
    ("identb", [128, 128], BF), ("identf", [128, 128], F32),
    ("i16", [16, 16], F32), ("m16", [16, 16], F32),
    ("signs4", [1, 4], F32), ("orthoc", [1, 4], F32),
    ("onesrowf", [1, 128], F32), ("ones16b", [16, 1], BF),
]


def _fold_weights(w):
    g = {k: np.asarray(v, np.float32) for k, v in w.items()}
    out = {}

    def put_hl(name, a):
        h, l = _hilo(a)
        out[name + "_h"], out[name + "_l"] = h, l

    wstk1 = np.concatenate([g["W1"], g["W1"] @ g["Wrw"], g["W1"] @ g["Wct"]], axis=1)
    put_hl("wstk1", wstk1)
    out["bstk1"] = np.concatenate(
        [g["b1"], g["b1"] @ g["Wrw"] + g["brw"], g["b1"] @ g["Wct"] + g["bct"]])[None, :]
    put_hl("grel48", np.concatenate([g["Wg_rel"], g["Wg_rel"] @ g["Wmcg"]], axis=1))
    put_hl("groot48", np.concatenate([g["Wg_root"], g["Wg_root"] @ g["Wmcg"]], axis=1))
    out["bg48"] = np.concatenate([g["bg"], g["bg"] @ g["Wmcg"] + g["bmcg"]])[None, :]
    put_hl("croot48", np.concatenate([g["Wc_root"], g["Wc_root"] @ g["Wmcc"]], axis=1))
    out["bc48"] = np.concatenate([g["bc"], g["bc"] @ g["Wmcc"] + g["bmcc"]])[None, :]
    out["ct48"] = _bf(np.concatenate([g["Wc_rel"], g["Wc_rel"] @ g["Wmcc"]], axis=1))
    put_hl("w2grel", g["W2g_rel"])
    put_hl("w2groot", g["W2g_root"])
    out["b2gcol"] = g["b2g"][:, None]
    put_hl("wcatg", g["Wcat"][0:32])
    put_hl("wcatc", g["Wcat"][32:64])
    out["bcat"] = g["bcat"][None, :]
    put_hl("w2", g["W2"])
    out["b2col"] = g["b2"][:, None]
    put_hl("w3", g["W3"])
    out["b3col"] = g["b3"][:, None]
    out["identb"] = _bf(np.eye(128))
    out["identf"] = np.eye(128, dtype=np.float32)
    out["i16"] = np.eye(16, dtype=np.float32)
    out["m16"] = (1.0 - np.eye(16)).astype(np.float32)
    out["signs4"] = np.array([[-1.0, 1.0, -1.0, -1.0]], np.float32)
    out["orthoc"] = np.array([[-2.0 / np.sqrt(2.0), -2.0 / np.sqrt(32.0),
                               -2.0 / np.sqrt(16.0), -2.0 / np.sqrt(16.0)]], np.float32)
    out["onesrowf"] = np.ones((1, 128), np.float32)
    out["ones16b"] = _bf(np.ones((16, 1)))
    return out


def _bcast(ap, parts):
    return bass.AP(tensor=ap.tensor, offset=ap.offset, ap=[[0, parts]] + list(ap.ap[1:]))


def _rep(ap2, w):
    return bass.AP(tensor=ap2.tensor, offset=ap2.offset, ap=list(ap2.ap) + [[0, w]])


def _r1(t):
    """[P, C] AP -> [P, 1, C] (for full-free-dim reduces)."""
    return t[:].rearrange("p (a c) -> p a c", a=1)


def build_nc():
    nc = bacc.Bacc(None)
    adj = nc.dram_tensor("adj", [GPC, N, N], F32, kind="ExternalInput")
    xth = nc.dram_tensor("xth", [GPC, 128, N], BF, kind="ExternalInput")
    xtl = nc.dram_tensor("xtl", [GPC, 128, N], BF, kind="ExternalInput")
    W = {}
    for name, shape, dt in WSPECS:
        W[name] = nc.dram_tensor(name, shape, dt, kind="ExternalInput")
    out_ls = nc.dram_tensor("out_ls", [GPC, NOUT], F32, kind="ExternalOutput")
    out_loss = nc.dram_tensor("out_loss", [GPC, 2], F32, kind="ExternalOutput")
    dbg = nc.dram_tensor("dbg", [128, 2048], F32, kind="ExternalOutput") if DEBUG else None

    with contextlib.ExitStack() as ctx:
        tc = ctx.enter_context(TileContext(nc))
        wp = ctx.enter_context(tc.tile_pool(name="wp", bufs=1))
        mp = ctx.enter_context(tc.tile_pool(name="mp", bufs=2))
        sp = ctx.enter_context(tc.tile_pool(name="sp", bufs=2))
        # PSUM budget (8 banks): pav x1 + pd2 x2 + pgc x1 + pwide(2bk) x1 + ptr x2
        pp = ctx.enter_context(tc.tile_pool(name="pp", bufs=2, space="PSUM"))
        p1 = ctx.enter_context(tc.tile_pool(name="p1", bufs=1, space="PSUM"))

        wt = {}
        for name, shape, dt in WSPECS:
            t = wp.tile(shape, dt, tag=name, name=name)
            nc.sync.dma_start(out=t[:], in_=W[name][:])
            wt[name] = t
        bstk1b = wp.tile([128, 66], F32, tag="bstk1b")
        nc.sync.dma_start(out=bstk1b[:], in_=_bcast(W["bstk1"][:], 128))
        bg48b = wp.tile([128, 48], F32, tag="bg48b")
        nc.sync.dma_start(out=bg48b[:], in_=_bcast(W["bg48"][:], 128))
        bc48b = wp.tile([128, 48], F32, tag="bc48b")
        nc.sync.dma_start(out=bc48b[:], in_=_bcast(W["bc48"][:], 128))
        bcatb = wp.tile([16, 32], F32, tag="bcatb")
        nc.sync.dma_start(out=bcatb[:], in_=_bcast(W["bcat"][:], 16))
        zb = wp.tile([128, 1], F32, tag="zb")
        nc.vector.memset(zb[:], 0.0)

        for g in range(GPC):
            _emit_graph(nc, wp, mp, sp, pp, p1, wt, bstk1b, bg48b, bc48b,
                        bcatb, zb, adj, xth, xtl, out_ls, out_loss, g, dbg)
    nc.finalize()
    return nc


def _emit_graph(nc, wp, mp, sp, pp, p1, wt, bstk1b, bg48b, bc48b, bcatb, zb,
                adj, xth, xtl, out_ls, out_loss, g, dbg=None):
    v, sc, te, gp = nc.vector, nc.scalar, nc.tensor, nc.gpsimd
    idb, idf = wt["identb"], wt["identf"]

    # ---- persistent per-graph tiles ----
    A = [mp.tile([128, N], BF, tag=f"abf{bi}", name=f"abf{bi}") for bi in range(NB)]
    CTA = [mp.tile([128, N], BF, tag=f"cta{bi}", name=f"cta{bi}", bufs=1) for bi in range(NB)]
    xthT = mp.tile([128, N], BF, tag="xthT")
    xtlT = mp.tile([128, N], BF, tag="xtlT")
    XEZ = mp.tile([128, NB * XC], F32, tag="XEZ", bufs=1)
    Vw = mp.tile([128, NB * VW], BF, tag="Vw", bufs=1)
    AVW = mp.tile([128, NB * 132], F32, tag="AVW", bufs=1)
    CGW = mp.tile([128, NB * 32], F32, tag="CGW", bufs=1)
    DW = mp.tile([128, NB * DPW], BF, tag="DW", bufs=1)
    DTL = mp.tile([34, N], BF, tag="DTL", bufs=1)
    DTR = mp.tile([34, N], BF, tag="DTR", bufs=1)
    XE1 = mp.tile([128, NB * 33], BF, tag="XE1")
    cgTh = mp.tile([32, N], BF, tag="cgTh", bufs=1)
    cgTl = mp.tile([32, N], BF, tag="cgTl", bufs=1)
    xeTh = mp.tile([32, N], BF, tag="xeTh", bufs=1)
    xeTl = mp.tile([32, N], BF, tag="xeTl", bufs=1)
    convT = mp.tile([33, N], BF, tag="convT", bufs=1)
    fW = mp.tile([128, 8], F32, tag="fW")
    f2W = mp.tile([128, 8], F32, tag="f2W")
    tfW = mp.tile([128, 8], F32, tag="tfW")
    sqW = mp.tile([128, 8], F32, tag="sqW")
    srw = mp.tile([128, 16], F32, tag="srw")
    sct = mp.tile([128, 256], F32, tag="sct")
    scth = mp.tile([128, 256], BF, tag="scth")
    sctl = mp.tile([128, 256], BF, tag="sctl")
    frep = mp.tile([128, 256], F32, tag="frep")
    f2rep = mp.tile([128, 256], F32, tag="f2rep")
    tfrep = mp.tile([128, 256], F32, tag="tfrep")
    dG = mp.tile([128, 8], F32, tag="dG")
    dwide = mp.tile([128, 8], F32, tag="dwide")
    LB = mp.tile([128, 11], F32, tag="LB")
    LBR = mp.tile([128, 11], F32, tag="LBR")
    LBR2 = mp.tile([128, 11], F32, tag="LBR2")
    xgh = mp.tile([128, NB * 32], BF, tag="xgh")
    xgl = mp.tile([128, NB * 32], BF, tag="xgl")
    xch = mp.tile([128, NB * 32], BF, tag="xch")
    xcl = mp.tile([128, NB * 32], BF, tag="xcl")
    ZG = mp.tile([128, NB * 16], F32, tag="ZG")
    ZC = mp.tile([128, NB * 16], F32, tag="ZC")
    PQR = mp.tile([128, NB * 48], BF, tag="PQR")
    s2gl = mp.tile([128, NB * 16], BF, tag="s2gl")
    S2CB = mp.tile([128, NB * 17], BF, tag="S2CB")
    s2cl = mp.tile([128, NB * 16], BF, tag="s2cl")
    nsqg = mp.tile([128, 8], F32, tag="nsqg")
    nsqc = mp.tile([128, 8], F32, tag="nsqc")
    ABn = mp.tile([128, NB * 48], BF, tag="ABn", bufs=1)
    CSn = mp.tile([128, NB * 17], BF, tag="CSn", bufs=1)
    catG = mp.tile([32, 16], BF, tag="catG")
    catGl = mp.tile([32, 16], BF, tag="catGl")
    catC = mp.tile([32, 16], BF, tag="catC")
    catCl = mp.tile([32, 16], BF, tag="catCl")
    LB16 = mp.tile([16, 2], F32, tag="LB16")
    LB16R = mp.tile([16, 2], F32, tag="LB16R")
    SCAL = mp.tile([1, 16], F32, tag="SCAL")

    def r3(t, c):
        return t[:].rearrange("p (b c) -> p b c", b=NB) if c is None else \
            t[:].rearrange("p (b c) -> p b c", c=c)

    XEZ3 = XEZ[:].rearrange("p (b c) -> p b c", b=NB)
    Vw3 = Vw[:].rearrange("p (b c) -> p b c", b=NB)
    AVW3 = AVW[:].rearrange("p (b c) -> p b c", b=NB)
    CGW3 = CGW[:].rearrange("p (b c) -> p b c", b=NB)
    DW3 = DW[:].rearrange("p (b c) -> p b c", b=NB)
    XE13 = XE1[:].rearrange("p (b c) -> p b c", b=NB)
    xgh3 = xgh[:].rearrange("p (b c) -> p b c", b=NB)
    xgl3 = xgl[:].rearrange("p (b c) -> p b c", b=NB)
    xch3 = xch[:].rearrange("p (b c) -> p b c", b=NB)
    xcl3 = xcl[:].rearrange("p (b c) -> p b c", b=NB)
    PQR3 = PQR[:].rearrange("p (b c) -> p b c", b=NB)
    S2CB3 = S2CB[:].rearrange("p (b c) -> p b c", b=NB)
    ABn3 = ABn[:].rearrange("p (b c) -> p b c", b=NB)
    CSn3 = CSn[:].rearrange("p (b c) -> p b c", b=NB)
    sct3 = sct[:].rearrange("p (b c) -> p b c", b=NB)
    scth3 = scth[:].rearrange("p (b c) -> p b c", b=NB)
    sctl3 = sctl[:].rearrange("p (b c) -> p b c", b=NB)
    srw3 = srw[:].rearrange("p (b c) -> p b c", b=NB)
    s2gl3 = s2gl[:].rearrange("p (b c) -> p b c", b=NB)
    s2cl3 = s2cl[:].rearrange("p (b c) -> p b c", b=NB)
    ZG3 = ZG[:].rearrange("p (b c) -> p b c", b=NB)
    ZC3 = ZC[:].rearrange("p (b c) -> p b c", b=NB)
    frep3 = frep[:].rearrange("p (b c) -> p b c", b=NB)
    f2rep3 = f2rep[:].rearrange("p (b c) -> p b c", b=NB)
    tfrep3 = tfrep[:].rearrange("p (b c) -> p b c", b=NB)

    # ======== phase L: loads + A cast ========
    for bi in range(NB):
        stg = sp.tile([128, N], F32, tag="stg", bufs=2)
        nc.sync.dma_start(out=stg[:], in_=adj[g, bi * 128:(bi + 1) * 128, :])
        if bi % 2 == 0:
            v.tensor_copy(A[bi][:], stg[:])
        else:
            sc.copy(out=A[bi][:], in_=stg[:])
    nc.sync.dma_start(out=xthT[:], in_=xth[g, :, :])
    nc.sync.dma_start(out=xtlT[:], in_=xtl[g, :, :])

    # ======== phase E: x_emb + z-logits ========
    for bi in range(NB):
        sl = slice(bi * 128, (bi + 1) * 128)
        ps = pp.tile([128, 66], F32, tag="pav", bufs=1)
        te.matmul(ps[:], lhsT=xthT[:, sl], rhs=wt["wstk1_h"][:], start=True, stop=False)
        te.matmul(ps[:], lhsT=xthT[:, sl], rhs=wt["wstk1_l"][:], start=False, stop=False)
        te.matmul(ps[:], lhsT=xtlT[:, sl], rhs=wt["wstk1_h"][:], start=False, stop=True)
        v.tensor_tensor(out=XEZ[:, bi * XC:(bi + 1) * XC], in0=ps[:], in1=bstk1b[:], op=ALU.add)

    # ======== phase S: softmaxes, reps, dpack, V ========
    erw = sp.tile([128, 16], F32, tag="erw")
    sc.activation(out=r3(erw, 2), in_=XEZ3[:, :, 32:34], func=AF.Exp, bias=zb[:])
    rsum = sp.tile([128, 8], F32, tag="rsum")
    v.tensor_reduce(out=rsum[:], in_=r3(erw, 2), axis=AX.X, op=ALU.add)
    rrec = sp.tile([128, 8], F32, tag="rrec")
    v.reciprocal(rrec[:], rsum[:])
    v.tensor_tensor(out=srw3, in0=r3(erw, 2), in1=_rep(rrec[:], 2), op=ALU.mult)
    v.tensor_copy(fW[:].rearrange("p (b c) -> p b c", c=1), srw3[:, :, 0:1])
    v.tensor_tensor(out=f2W[:], in0=fW[:], in1=fW[:], op=ALU.mult)
    v.tensor_scalar(out=tfW[:], in0=fW[:], scalar1=2.0, scalar2=None, op0=ALU.mult)
    v.tensor_copy(frep3, _rep(fW[:], 32))
    v.tensor_copy(f2rep3, _rep(f2W[:], 32))
    v.tensor_copy(tfrep3, _rep(tfW[:], 32))
    ect = sp.tile([128, 256], F32, tag="ect")
    sc.activation(out=r3(ect, 32), in_=XEZ3[:, :, 34:66], func=AF.Exp, bias=zb[:])
    csum = sp.tile([128, 8], F32, tag="csum")
    v.tensor_reduce(out=csum[:], in_=r3(ect, 32), axis=AX.X, op=ALU.add)
    crec = sp.tile([128, 8], F32, tag="crec")
    v.reciprocal(crec[:], csum[:])
    v.tensor_tensor(out=sct3, in0=r3(ect, 32), in1=_rep(crec[:], 32), op=ALU.mult)
    v.tensor_copy(scth3, sct3)
    v.tensor_tensor(out=sctl3, in0=sct3, in1=scth3, op=ALU.subtract)
    sq2 = sp.tile([128, 256], F32, tag="sq2")
    v.tensor_tensor(out=sq2[:], in0=sct[:], in1=sct[:], op=ALU.mult)
    v.tensor_reduce(out=sqW[:], in_=r3(sq2, 32), axis=AX.X, op=ALU.add)
    # dpack
    v.tensor_copy(DW3[:, :, 0:32], sct3)
    v.tensor_copy(DW3[:, :, 32:33], sqW[:].rearrange("p (b c) -> p b c", c=1))
    v.tensor_copy(DW3[:, :, 67:68], sqW[:].rearrange("p (b c) -> p b c", c=1))
    v.memset(DW3[:, :, 33:34], 1.0)
    v.memset(DW3[:, :, 66:67], 1.0)
    v.tensor_scalar(out=DW3[:, :, 34:66], in0=sct3, scalar1=-2.0, scalar2=None, op0=ALU.mult)
    # V
    v.tensor_copy(Vw3[:, :, 0:32], XEZ3[:, :, 0:32])
    v.tensor_tensor(out=Vw3[:, :, 132:164], in0=XEZ3[:, :, 0:32], in1=Vw3[:, :, 0:32], op=ALU.subtract)
    fx = sp.tile([128, 256], F32, tag="fx")
    v.tensor_tensor(out=r3(fx, 32), in0=XEZ3[:, :, 0:32], in1=frep3, op=ALU.mult)
    v.tensor_copy(Vw3[:, :, 32:64], r3(fx, 32))
    v.tensor_tensor(out=Vw3[:, :, 164:196], in0=r3(fx, 32), in1=Vw3[:, :, 32:64], op=ALU.subtract)
    f2x = sp.tile([128, 256], F32, tag="f2x")
    v.tensor_tensor(out=r3(f2x, 32), in0=XEZ3[:, :, 0:32], in1=f2rep3, op=ALU.mult)
    v.tensor_copy(Vw3[:, :, 64:96], r3(f2x, 32))
    v.tensor_tensor(out=Vw3[:, :, 196:228], in0=r3(f2x, 32), in1=Vw3[:, :, 64:96], op=ALU.subtract)
    v.tensor_copy(Vw3[:, :, 96:98], srw3)
    v.tensor_tensor(out=Vw3[:, :, 228:230], in0=srw3, in1=Vw3[:, :, 96:98], op=ALU.subtract)
    v.tensor_copy(Vw3[:, :, 98:130], scth3)
    v.tensor_copy(Vw3[:, :, 230:262], sctl3)
    v.memset(Vw3[:, :, 130:131], 1.0)
    v.tensor_copy(Vw3[:, :, 131:132], f2W[:].rearrange("p (b c) -> p b c", c=1))
    v.memset(Vw3[:, :, 262:264], 0.0)
    v.tensor_copy(XE13[:, :, 0:32], XEZ3[:, :, 0:32])
    v.memset(XE13[:, :, 32:33], 1.0)

    # ======== phase D2: transpose dpack, d2, ct, ctA ========
    for bi in range(NB):
        psTa = p1.tile([34, 128], BF, tag="ptr")
        te.transpose(psTa[:], DW[:, bi * DPW:bi * DPW + 34], idb[:, :])
        sc.copy(out=DTL[:, bi * 128:(bi + 1) * 128], in_=psTa[:])
        psTb = p1.tile([34, 128], BF, tag="ptr")
        te.transpose(psTb[:], DW[:, bi * DPW + 34:(bi + 1) * DPW], idb[:, :])
        sc.copy(out=DTR[:, bi * 128:(bi + 1) * 128], in_=psTb[:])
    for bi in range(NB):
        for ch in range(2):
            psD = pp.tile([128, 512], F32, tag="pd2")
            te.matmul(psD[:], lhsT=DTL[:, bi * 128:(bi + 1) * 128],
                      rhs=DTR[:, ch * 512:(ch + 1) * 512], start=True, stop=True)
            cl = sp.tile([128, 512], BF, tag="cl")
            v.tensor_scalar(out=cl[:], in0=psD[:], scalar1=0.0, scalar2=SQRT_BIAS,
                            op0=ALU.max, op1=ALU.add)
            ctt = sp.tile([128, 512], BF, tag="ctt")
            sc.activation(out=ctt[:], in_=cl[:], func=AF.Sqrt, bias=zb[:])
            v.tensor_tensor(out=CTA[bi][:, ch * 512:(ch + 1) * 512], in0=ctt[:],
                            in1=A[bi][:, ch * 512:(ch + 1) * 512], op=ALU.mult)

    # ======== phase A: AV = A @ (Vh + Vl) ========
    for bi in range(NB):
        ps = pp.tile([128, 264], F32, tag="pav", bufs=1)
        for bj in range(NB):
            te.matmul(ps[:], lhsT=A[bj][:, bi * 128:(bi + 1) * 128],
                      rhs=Vw[:, bj * VW:(bj + 1) * VW],
                      start=(bj == 0), stop=(bj == NB - 1))
        avt = sp.tile([128, 132], F32, tag="avt")
        sc.copy(out=avt[:], in_=ps[:, 132:264])
        v.tensor_tensor(out=AVW[:, bi * 132:(bi + 1) * 132], in0=ps[:, 0:132],
                        in1=avt[:], op=ALU.add)

    # ======== phase C: conv_gap + dG ========
    u1 = sp.tile([128, 256], F32, tag="u1")
    u2 = sp.tile([128, 256], F32, tag="u2")
    v.tensor_tensor(out=r3(u1, 32), in0=AVW3[:, :, 0:32], in1=f2rep3, op=ALU.mult)
    v.tensor_tensor(out=r3(u2, 32), in0=AVW3[:, :, 0:32], in1=r3(u1, 32), op=ALU.subtract)
    v.tensor_tensor(out=r3(u1, 32), in0=AVW3[:, :, 32:64], in1=tfrep3, op=ALU.mult)
    v.tensor_tensor(out=r3(u2, 32), in0=r3(u2, 32), in1=r3(u1, 32), op=ALU.add)
    v.tensor_tensor(out=CGW3, in0=r3(u2, 32), in1=AVW3[:, :, 64:96], op=ALU.subtract)
    v.tensor_copy(dwide[:].rearrange("p (b c) -> p b c", c=1), AVW3[:, :, 130:131])
    w1t = sp.tile([128, 8], F32, tag="w1t")
    w2t = sp.tile([128, 8], F32, tag="w2t")
    v.tensor_tensor(out=w1t[:], in0=dwide[:], in1=f2W[:], op=ALU.mult)
    v.tensor_tensor(out=w2t[:], in0=dwide[:], in1=w1t[:], op=ALU.subtract)
    v.tensor_tensor(out=w1t[:].rearrange("p (b c) -> p b c", c=1),
                    in0=AVW3[:, :, 96:97], in1=tfW[:].rearrange("p (b c) -> p b c", c=1),
                    op=ALU.mult)
    v.tensor_tensor(out=w2t[:], in0=w2t[:], in1=w1t[:], op=ALU.add)
    v.tensor_tensor(out=dG[:].rearrange("p (b c) -> p b c", c=1),
                    in0=w2t[:].rearrange("p (b c) -> p b c", c=1),
                    in1=AVW3[:, :, 131:132], op=ALU.subtract)

    # ======== phase LOSS-A: per-node loss columns ========
    # LB cols: 0 num_rw | 1 den_rw | 2 sds | 3 sas | 4 den_ct/tr_ct | 5 vol |
    #          6 den1 | 7 tr_rw | 8 tr2g | 9 tr2c
    t16 = sp.tile([128, 16], F32, tag="t16")
    v.tensor_tensor(out=r3(t16, 2), in0=srw3, in1=AVW3[:, :, 96:98], op=ALU.mult)
    v.tensor_reduce(out=LB[:, 0:1], in_=_r1(t16), axis=AX.X, op=ALU.add)
    t8 = sp.tile([128, 8], F32, tag="t8")
    v.tensor_tensor(out=r3(t16, 2), in0=srw3, in1=srw3, op=ALU.mult)
    v.tensor_reduce(out=t8[:], in_=r3(t16, 2), axis=AX.X, op=ALU.add)
    v.tensor_reduce(out=LB[:, 7:8], in_=_r1(t8), axis=AX.X, op=ALU.add)
    v.tensor_tensor(out=t8[:], in0=t8[:], in1=dwide[:], op=ALU.mult)
    v.tensor_reduce(out=LB[:, 1:2], in_=_r1(t8), axis=AX.X, op=ALU.add)
    v.tensor_tensor(out=t8[:], in0=sqW[:], in1=dwide[:], op=ALU.mult)
    v.tensor_reduce(out=LB[:, 2:3], in_=_r1(t8), axis=AX.X, op=ALU.add)
    t256 = sp.tile([128, 256], F32, tag="t256")
    v.tensor_tensor(out=r3(t256, 32), in0=sct3, in1=AVW3[:, :, 98:130], op=ALU.mult)
    v.tensor_reduce(out=LB[:, 3:4], in_=_r1(t256), axis=AX.X, op=ALU.add)
    v.tensor_reduce(out=LB[:, 4:5], in_=_r1(sqW), axis=AX.X, op=ALU.add)
    v.tensor_reduce(out=LB[:, 5:6], in_=_r1(dwide), axis=AX.X, op=ALU.add)

    # ======== phase CT-A: convT = ([x|1]^T ctA) * iv ========
    psCV = p1.tile([33, N], F32, tag="pwide")
    for bj in range(NB):
        for ch in range(2):
            te.matmul(psCV[:, ch * 512:(ch + 1) * 512],
                      lhsT=XE1[:, bj * 33:(bj + 1) * 33],
                      rhs=CTA[bj][:, ch * 512:(ch + 1) * 512],
                      start=(bj == 0), stop=(bj == NB - 1))
    gp.partition_all_reduce(LBR[:], LB[:], channels=128, reduce_op=RED.add)
    v.tensor_scalar(out=SCAL[:, 0:1], in0=LBR[0:1, 5:6], scalar1=float(EPS),
                    scalar2=None, op0=ALU.add)
    v.reciprocal(SCAL[:, 1:2], SCAL[:, 0:1])
    psiv = p1.tile([33, 1], F32, tag="ptr")
    te.matmul(psiv[:], lhsT=wt["onesrowf"][0:1, 0:33], rhs=SCAL[0:1, 1:2],
              start=True, stop=True)
    ivb = sp.tile([33, 1], F32, tag="ivb")
    v.tensor_copy(ivb[:], psiv[:])
    v.tensor_scalar(out=convT[:], in0=psCV[:], scalar1=ivb[:], scalar2=None, op0=ALU.mult)

    # ======== phase P2: [conv_gapT ; x_embT] hi/lo ========
    for bi in range(NB):
        sl = slice(bi * 128, (bi + 1) * 128)
        psP1 = p1.tile([32, 128], F32, tag="ptr")
        te.transpose(psP1[:], CGW[:, bi * 32:(bi + 1) * 32], idf[:, :])
        v.tensor_copy(cgTh[:, sl], psP1[:])
        v.tensor_tensor(out=cgTl[:, sl], in0=psP1[:], in1=cgTh[:, sl], op=ALU.subtract)
        psP2 = p1.tile([32, 128], F32, tag="ptr")
        te.transpose(psP2[:], XEZ3[:, bi, 0:32], idf[:, :])
        v.tensor_copy(xeTh[:, sl], psP2[:])
        v.tensor_tensor(out=xeTl[:, sl], in0=psP2[:], in1=xeTh[:, sl], op=ALU.subtract)

    # ======== phase XG: x_gap/z2g and x_ct/z2c ========
    for bi in range(NB):
        sl = slice(bi * 128, (bi + 1) * 128)
        psG = pp.tile([128, 48], F32, tag="pgc")
        psC = pp.tile([128, 48], F32, tag="pgc")
        te.matmul(psG[:], lhsT=cgTh[:, sl], rhs=wt["grel48_h"][:], start=True, stop=False)
        te.matmul(psG[:], lhsT=cgTh[:, sl], rhs=wt["grel48_l"][:], start=False, stop=False)
        te.matmul(psG[:], lhsT=cgTl[:, sl], rhs=wt["grel48_h"][:], start=False, stop=False)
        te.matmul(psG[:], lhsT=xeTh[:, sl], rhs=wt["groot48_h"][:], start=False, stop=False)
        te.matmul(psG[:], lhsT=xeTh[:, sl], rhs=wt["groot48_l"][:], start=False, stop=False)
        te.matmul(psC[:], lhsT=xeTh[:, sl], rhs=wt["croot48_h"][:], start=True, stop=False)
        te.matmul(psC[:], lhsT=xeTh[:, sl], rhs=wt["croot48_l"][:], start=False, stop=False)
        te.matmul(psG[:], lhsT=xeTl[:, sl], rhs=wt["groot48_h"][:], start=False, stop=True)
        te.matmul(psC[:], lhsT=xeTl[:, sl], rhs=wt["croot48_h"][:], start=False, stop=False)
        te.matmul(psC[:], lhsT=convT[0:32, sl], rhs=wt["ct48"][:], start=False, stop=True)
        ev = sp.tile([128, 48], F32, tag="evG")
        v.tensor_tensor(out=ev[:], in0=psG[:], in1=bg48b[:], op=ALU.add)
        v.tensor_copy(xgh3[:, bi, :], ev[:, 0:32])
        v.tensor_tensor(out=xgl3[:, bi, :], in0=ev[:, 0:32], in1=xgh3[:, bi, :], op=ALU.subtract)
        v.tensor_copy(ZG3[:, bi, :], ev[:, 32:48])
        ev2 = sp.tile([128, 48], F32, tag="evC")
        v.tensor_tensor(out=ev2[:], in0=psC[:], in1=bc48b[:], op=ALU.add)
        v.tensor_copy(xch3[:, bi, :], ev2[:, 0:32])
        v.tensor_tensor(out=xcl3[:, bi, :], in0=ev2[:, 0:32], in1=xch3[:, bi, :], op=ALU.subtract)
        v.tensor_copy(ZC3[:, bi, :], ev2[:, 32:48])

    # ======== phase S2: pool softmaxes ========
    for which in ("g", "c"):
        Z3 = ZG3 if which == "g" else ZC3
        e2 = sp.tile([128, 128], F32, tag="e2")
        sc.activation(out=r3(e2, 16), in_=Z3, func=AF.Exp, bias=zb[:])
        zsum = sp.tile([128, 8], F32, tag="zsum")
        v.tensor_reduce(out=zsum[:], in_=r3(e2, 16), axis=AX.X, op=ALU.add)
        zrec = sp.tile([128, 8], F32, tag="zrec")
        v.reciprocal(zrec[:], zsum[:])
        s32 = sp.tile([128, 128], F32, tag="s32")
        v.tensor_tensor(out=r3(s32, 16), in0=r3(e2, 16), in1=_rep(zrec[:], 16), op=ALU.mult)
        n2 = sp.tile([128, 128], F32, tag="n2")
        v.tensor_tensor(out=r3(n2, 16), in0=r3(s32, 16), in1=r3(s32, 16), op=ALU.mult)
        nsq = nsqg if which == "g" else nsqc
        v.tensor_reduce(out=nsq[:], in_=r3(n2, 16), axis=AX.X, op=ALU.add)
        lbc = 8 if which == "g" else 9
        v.tensor_reduce(out=LB[:, lbc:lbc + 1], in_=_r1(nsq), axis=AX.X, op=ALU.add)
        if which == "g":
            v.tensor_copy(PQR3[:, :, 0:16], r3(s32, 16))
            v.tensor_tensor(out=PQR3[:, :, 16:32], in0=r3(s32, 16), in1=_rep(fW[:], 16), op=ALU.mult)
            v.tensor_tensor(out=PQR3[:, :, 32:48], in0=r3(s32, 16), in1=_rep(f2W[:], 16), op=ALU.mult)
            v.tensor_tensor(out=s2gl3, in0=r3(s32, 16), in1=PQR3[:, :, 0:16], op=ALU.subtract)
            t8b = sp.tile([128, 8], F32, tag="t8b")
            v.tensor_tensor(out=t8b[:], in0=nsq[:], in1=dG[:], op=ALU.mult)
            v.tensor_reduce(out=LB[:, 6:7], in_=_r1(t8b), axis=AX.X, op=ALU.add)
        else:
            v.tensor_copy(S2CB3[:, :, 0:16], r3(s32, 16))
            v.memset(S2CB3[:, :, 16:17], 1.0)
            v.tensor_tensor(out=s2cl3, in0=r3(s32, 16), in1=S2CB3[:, :, 0:16], op=ALU.subtract)

    # ======== phase B-GAP ========
    psAB = p1.tile([48, N], F32, tag="pwide")
    for bj in range(NB):
        for ch in range(2):
            te.matmul(psAB[:, ch * 512:(ch + 1) * 512],
                      lhsT=PQR[:, bj * 48:(bj + 1) * 48],
                      rhs=A[bj][:, ch * 512:(ch + 1) * 512],
                      start=(bj == 0), stop=(bj == NB - 1))
    ABt = sp.tile([48, N], BF, tag="ABt")
    sc.copy(out=ABt[:, 0:512], in_=psAB[:, 0:512])
    sc.copy(out=ABt[:, 512:1024], in_=psAB[:, 512:1024])
    for bi in range(NB):
        psn = p1.tile([128, 48], BF, tag="ptr")
        te.transpose(psn[:], ABt[:, bi * 128:(bi + 1) * 128], idb[0:48, 0:48])
        v.tensor_copy(ABn3[:, bi, :], psn[:])
    psOa = p1.tile([16, 48], F32, tag="ptr")
    for bi in range(NB):
        te.matmul(psOa[:], lhsT=PQR3[:, bi, 0:16], rhs=ABn3[:, bi, 0:48],
                  start=(bi == 0), stop=(bi == NB - 1))
    oPP = sp.tile([16, 48], F32, tag="oPP")
    sc.copy(out=oPP[:], in_=psOa[:])
    psOb = p1.tile([16, 16], F32, tag="ptr")
    for bi in range(NB):
        te.matmul(psOb[:], lhsT=PQR3[:, bi, 32:48], rhs=ABn3[:, bi, 0:16],
                  start=(bi == 0), stop=(bi == NB - 1))
    oag = sp.tile([16, 16], F32, tag="oag")
    v.tensor_tensor(out=oag[:], in0=oPP[:, 0:16], in1=psOb[:], op=ALU.subtract)
    v.tensor_tensor(out=oag[:], in0=oag[:], in1=oPP[:, 32:48], op=ALU.subtract)
    psOc = p1.tile([16, 16], F32, tag="ptr")
    for bi in range(NB):
        te.matmul(psOc[:], lhsT=PQR3[:, bi, 16:32], rhs=ABn3[:, bi, 16:32],
                  start=(bi == 0), stop=(bi == NB - 1))
    t2q = sp.tile([16, 16], F32, tag="t2q")
    v.tensor_scalar(out=t2q[:], in0=psOc[:], scalar1=2.0, scalar2=None, op0=ALU.mult)
    v.tensor_tensor(out=oag[:], in0=oag[:], in1=t2q[:], op=ALU.add)

    # ======== phase B-CT ========
    psCS = p1.tile([17, N], F32, tag="pwide")
    for bj in range(NB):
        for ch in range(2):
            te.matmul(psCS[:, ch * 512:(ch + 1) * 512],
                      lhsT=S2CB[:, bj * 17:(bj + 1) * 17],
                      rhs=CTA[bj][:, ch * 512:(ch + 1) * 512],
                      start=(bj == 0), stop=(bj == NB - 1))
    CSt = sp.tile([17, N], BF, tag="CSt")
    sc.copy(out=CSt[:, 0:512], in_=psCS[:, 0:512])
    sc.copy(out=CSt[:, 512:1024], in_=psCS[:, 512:1024])
    for bi in range(NB):
        psn2 = p1.tile([128, 17], BF, tag="ptr")
        te.transpose(psn2[:], CSt[:, bi * 128:(bi + 1) * 128], idb[0:17, 0:17])
        v.tensor_copy(CSn3[:, bi, :], psn2[:])
    # den2 = sum_n (ctA@1)_n * |s2c_n|^2  (unscaled), all in normal layout
    dctn = sp.tile([128, 8], F32, tag="dctn")
    v.tensor_copy(dctn[:].rearrange("p (b c) -> p b c", c=1), CSn3[:, :, 16:17])
    v.tensor_tensor(out=dctn[:], in0=dctn[:], in1=nsqc[:], op=ALU.mult)
    v.tensor_reduce(out=LB[:, 10:11], in_=_r1(dctn), axis=AX.X, op=ALU.add)
    psOC = p1.tile([16, 16], F32, tag="ptr")
    for bi in range(NB):
        te.matmul(psOC[:], lhsT=S2CB3[:, bi, 0:16], rhs=CSn3[:, bi, 0:16],
                  start=(bi == 0), stop=(bi == NB - 1))
    oac = sp.tile([16, 16], F32, tag="oac")
    v.tensor_copy(oac[:], psOC[:])

    tdi = sp.tile([16, 16], F32, tag="tdi")
    v.tensor_tensor(out=tdi[:], in0=oag[:], in1=wt["i16"][:], op=ALU.mult)
    v.tensor_reduce(out=LB16[:, 0:1], in_=_r1(tdi), axis=AX.X, op=ALU.add)
    v.tensor_tensor(out=tdi[:], in0=oac[:], in1=wt["i16"][:], op=ALU.mult)
    v.tensor_reduce(out=LB16[:, 1:2], in_=_r1(tdi), axis=AX.X, op=ALU.add)

    # ======== pooled branches -> catT ========
    for which in ("g", "c"):
        oam = oag if which == "g" else oac

        xh3_ = xgh3 if which == "g" else xch3
        xl3_ = xgl3 if which == "g" else xcl3
        prh3 = PQR3[:, :, 0:16] if which == "g" else S2CB3[:, :, 0:16]
        prl3 = s2gl3 if which == "g" else s2cl3
        oa = sp.tile([16, 16], F32, tag="oa")
        v.tensor_tensor(out=oa[:], in0=oam[:], in1=wt["m16"][:], op=ALU.mult)
        rs = sp.tile([16, 1], F32, tag="rs")
        v.tensor_reduce(out=rs[:], in_=_r1(oa), axis=AX.X, op=ALU.add)
        dd = sp.tile([16, 1], F32, tag="dd")
        sc.activation(out=dd[:], in_=rs[:], func=AF.Sqrt, bias=zb[0:16, :])
        v.tensor_scalar(out=dd[:], in0=dd[:], scalar1=float(EPS), scalar2=None, op0=ALU.add)
        rcd = sp.tile([16, 1], F32, tag="rcd")
        v.reciprocal(rcd[:], dd[:])
        t1s = sp.tile([16, 16], BF, tag="t1s")
        v.tensor_scalar(out=t1s[:], in0=oa[:], scalar1=rcd[:], scalar2=None, op0=ALU.mult)
        pst = p1.tile([16, 16], BF, tag="ptr")
        te.transpose(pst[:], t1s[:], idb[0:16, 0:16])
        apg = sp.tile([16, 16], F32, tag="apg")
        v.tensor_scalar(out=apg[:], in0=pst[:], scalar1=rcd[:], scalar2=None, op0=ALU.mult)
        apgh = sp.tile([16, 16], BF, tag="apgh")
        v.tensor_copy(apgh[:], apg[:])
        apgl = sp.tile([16, 16], BF, tag="apgl")
        v.tensor_tensor(out=apgl[:], in0=apg[:], in1=apgh[:], op=ALU.subtract)
        psPP = p1.tile([32, 16], F32, tag="ptr")
        for bi in range(NB):
            st, last = (bi == 0), (bi == NB - 1)
            te.matmul(psPP[:], lhsT=xh3_[:, bi, :], rhs=prh3[:, bi, :], start=st, stop=False)
            te.matmul(psPP[:], lhsT=xl3_[:, bi, :], rhs=prh3[:, bi, :], start=False, stop=False)
            te.matmul(psPP[:], lhsT=xh3_[:, bi, :], rhs=prl3[:, bi, :], start=False, stop=last)
        poolT = sp.tile([32, 16], F32, tag="poolT")
        sc.copy(out=poolT[:], in_=psPP[:])
        poolTh = sp.tile([32, 16], BF, tag="poolTh")
        v.tensor_copy(poolTh[:], poolT[:])
        poolTl = sp.tile([32, 16], BF, tag="poolTl")
        v.tensor_tensor(out=poolTl[:], in0=poolT[:], in1=poolTh[:], op=ALU.subtract)
        psPN = p1.tile([16, 32], BF, tag="ptr")
        te.transpose(psPN[:], poolTh[:], idb[0:32, 0:32])
        poolNh = sp.tile([16, 32], BF, tag="poolNh")
        v.tensor_copy(poolNh[:], psPN[:])
        psPN2 = p1.tile([16, 32], BF, tag="ptr")
        te.transpose(psPN2[:], poolTl[:], idb[0:32, 0:32])
        poolNl = sp.tile([16, 32], BF, tag="poolNl")
        v.tensor_copy(poolNl[:], psPN2[:])
        psT1 = p1.tile([32, 16], F32, tag="ptr")
        te.matmul(psT1[:], lhsT=poolNh[:], rhs=apgh[:], start=True, stop=False)
        te.matmul(psT1[:], lhsT=poolNl[:], rhs=apgh[:], start=False, stop=False)
        te.matmul(psT1[:], lhsT=poolNh[:], rhs=apgl[:], start=False, stop=True)
        t1T = sp.tile([32, 16], F32, tag="t1T")
        v.tensor_copy(t1T[:], psT1[:])
        t1Th = sp.tile([32, 16], BF, tag="t1Th")
        v.tensor_copy(t1Th[:], t1T[:])
        t1Tl = sp.tile([32, 16], BF, tag="t1Tl")
        v.tensor_tensor(out=t1Tl[:], in0=t1T[:], in1=t1Th[:], op=ALU.subtract)
        psX2 = p1.tile([32, 16], F32, tag="ptr")
        te.matmul(psX2[:], lhsT=wt["w2grel_h"][:], rhs=t1Th[:], start=True, stop=False)
        te.matmul(psX2[:], lhsT=wt["w2grel_l"][:], rhs=t1Th[:], start=False, stop=False)
        te.matmul(psX2[:], lhsT=wt["w2grel_h"][:], rhs=t1Tl[:], start=False, stop=False)
        te.matmul(psX2[:], lhsT=wt["w2groot_h"][:], rhs=poolTh[:], start=False, stop=False)
        te.matmul(psX2[:], lhsT=wt["w2groot_l"][:], rhs=poolTh[:], start=False, stop=False)
        te.matmul(psX2[:], lhsT=wt["w2groot_h"][:], rhs=poolTl[:], start=False, stop=True)
        ex2 = sp.tile([32, 16], F32, tag="ex2")
        v.tensor_scalar(out=ex2[:], in0=psX2[:], scalar1=wt["b2gcol"][:], scalar2=None, op0=ALU.add)
        cth = catG if which == "g" else catC
        ctl = catGl if which == "g" else catCl
        v.tensor_copy(cth[:], ex2[:])
        v.tensor_tensor(out=ctl[:], in0=ex2[:], in1=cth[:], op=ALU.subtract)

    # ======== readout ========
    psM = p1.tile([16, 32], F32, tag="ptr")
    te.matmul(psM[:], lhsT=catG[:], rhs=wt["wcatg_h"][:], start=True, stop=False)
    te.matmul(psM[:], lhsT=catG[:], rhs=wt["wcatg_l"][:], start=False, stop=False)
    te.matmul(psM[:], lhsT=catGl[:], rhs=wt["wcatg_h"][:], start=False, stop=False)
    te.matmul(psM[:], lhsT=catC[:], rhs=wt["wcatc_h"][:], start=False, stop=False)
    te.matmul(psM[:], lhsT=catC[:], rhs=wt["wcatc_l"][:], start=False, stop=False)
    te.matmul(psM[:], lhsT=catCl[:], rhs=wt["wcatc_h"][:], start=False, stop=True)
    Mr = sp.tile([16, 32], F32, tag="Mr")
    v.tensor_tensor(out=Mr[:], in0=psM[:], in1=bcatb[:], op=ALU.add)
    v.tensor_scalar(out=Mr[:], in0=Mr[:], scalar1=0.0, scalar2=None, op0=ALU.max)
    Mrh = sp.tile([16, 32], BF, tag="Mrh")
    v.tensor_copy(Mrh[:], Mr[:])
    Mrl = sp.tile([16, 32], BF, tag="Mrl")
    v.tensor_tensor(out=Mrl[:], in0=Mr[:], in1=Mrh[:], op=ALU.subtract)
    psH = p1.tile([32, 1], F32, tag="ptr")
    te.matmul(psH[:], lhsT=Mrh[:], rhs=wt["ones16b"][:], start=True, stop=False)
    te.matmul(psH[:], lhsT=Mrl[:], rhs=wt["ones16b"][:], start=False, stop=True)
    hT = sp.tile([32, 1], F32, tag="hT")
    v.tensor_copy(hT[:], psH[:])
    hTh = sp.tile([32, 1], BF, tag="hTh")
    v.tensor_copy(hTh[:], hT[:])
    hTl = sp.tile([32, 1], BF, tag="hTl")
    v.tensor_tensor(out=hTl[:], in0=hT[:], in1=hTh[:], op=ALU.subtract)
    psY = p1.tile([32, 1], F32, tag="ptr")
    te.matmul(psY[:], lhsT=wt["w2_h"][:], rhs=hTh[:], start=True, stop=False)
    te.matmul(psY[:], lhsT=wt["w2_l"][:], rhs=hTh[:], start=False, stop=False)
    te.matmul(psY[:], lhsT=wt["w2_h"][:], rhs=hTl[:], start=False, stop=True)
    yT = sp.tile([32, 1], F32, tag="yT")
    sc.activation(out=yT[:], in_=psY[:], func=AF.Relu, bias=wt["b2col"][:])
    yTh = sp.tile([32, 1], BF, tag="yTh")
    v.tensor_copy(yTh[:], yT[:])
    yTl = sp.tile([32, 1], BF, tag="yTl")
    v.tensor_tensor(out=yTl[:], in0=yT[:], in1=yTh[:], op=ALU.subtract)
    psL = p1.tile([10, 1], F32, tag="ptr")
    te.matmul(psL[:], lhsT=wt["w3_h"][:], rhs=yTh[:], start=True, stop=False)
    te.matmul(psL[:], lhsT=wt["w3_l"][:], rhs=yTh[:], start=False, stop=False)
    te.matmul(psL[:], lhsT=wt["w3_h"][:], rhs=yTl[:], start=False, stop=True)
    lgT = sp.tile([10, 1], F32, tag="lgT")
    v.tensor_scalar(out=lgT[:], in0=psL[:], scalar1=wt["b3col"][:], scalar2=None, op0=ALU.add)
    lgTh = sp.tile([10, 1], BF, tag="lgTh")
    v.tensor_copy(lgTh[:], lgT[:])
    lgTl = sp.tile([10, 1], BF, tag="lgTl")
    v.tensor_tensor(out=lgTl[:], in0=lgT[:], in1=lgTh[:], op=ALU.subtract)
    psLr = p1.tile([1, 10], F32, tag="ptr")
    te.matmul(psLr[:], lhsT=lgTh[:], rhs=idb[0:10, 0:10], start=True, stop=False)
    te.matmul(psLr[:], lhsT=lgTl[:], rhs=idb[0:10, 0:10], start=False, stop=True)
    lrow = sp.tile([1, 10], F32, tag="lrow")
    v.tensor_copy(lrow[:], psLr[:])
    mx = sp.tile([1, 1], F32, tag="mx")
    v.tensor_reduce(out=mx[:], in_=_r1(lrow), axis=AX.X, op=ALU.max)
    shr = sp.tile([1, 10], F32, tag="shr")
    v.tensor_scalar(out=shr[:], in0=lrow[:], scalar1=mx[:], scalar2=None, op0=ALU.subtract)
    eex = sp.tile([1, 10], F32, tag="eex")
    sume = sp.tile([1, 1], F32, tag="sume")
    sc.activation(out=eex[:], in_=shr[:], func=AF.Exp, bias=zb[0:1, :], accum_out=sume[:])
    lse = sp.tile([1, 1], F32, tag="lse")
    sc.activation(out=lse[:], in_=sume[:], func=AF.Ln, bias=zb[0:1, :])
    lsrow = sp.tile([1, 10], F32, tag="lsrow")
    v.tensor_scalar(out=lsrow[:], in0=shr[:], scalar1=lse[:], scalar2=None, op0=ALU.subtract)
    nc.sync.dma_start(out=out_ls[g:g + 1, :], in_=lsrow[:])

    # ======== losses ========
    # ss matrices in one PSUM bank: cols s_rw 0:2 | ct_h 2:34 | ct_l 34:66 |
    # ss2g 66:82 | ss2c 82:98
    fro = sp.tile([32, 4], F32, tag="fro")
    v.memset(fro[:], 0.0)
    scr2 = sp.tile([32, 32], F32, tag="scr2")
    psS2 = p1.tile([2, 2], F32, tag="ptr")
    for bi in range(NB):
        te.matmul(psS2[:], lhsT=Vw3[:, bi, 96:98], rhs=Vw3[:, bi, 96:98],
                  start=(bi == 0), stop=(bi == NB - 1))
    sc.activation(out=scr2[0:2, 0:2], in_=psS2[:], func=AF.Square, bias=zb[0:2, :],
                  accum_out=fro[0:2, 0:1])
    psS32 = p1.tile([32, 32], F32, tag="ptr")
    for bi in range(NB):
        st, last = (bi == 0), (bi == NB - 1)
        te.matmul(psS32[:], lhsT=scth3[:, bi, :], rhs=scth3[:, bi, :], start=st, stop=False)
        te.matmul(psS32[:], lhsT=sctl3[:, bi, :], rhs=scth3[:, bi, :], start=False, stop=False)
        te.matmul(psS32[:], lhsT=scth3[:, bi, :], rhs=sctl3[:, bi, :], start=False, stop=last)
    sc.activation(out=scr2[:, :], in_=psS32[:], func=AF.Square, bias=zb[0:32, :],
                  accum_out=fro[:, 1:2])
    psSG = p1.tile([16, 16], F32, tag="ptr")
    for bi in range(NB):
        te.matmul(psSG[:], lhsT=PQR3[:, bi, 0:16], rhs=PQR3[:, bi, 0:16],
                  start=(bi == 0), stop=(bi == NB - 1))
    sc.activation(out=scr2[0:16, 0:16], in_=psSG[:], func=AF.Square,
                  bias=zb[0:16, :], accum_out=fro[0:16, 2:3])
    psSC = p1.tile([16, 16], F32, tag="ptr")
    for bi in range(NB):
        te.matmul(psSC[:], lhsT=S2CB3[:, bi, 0:16], rhs=S2CB3[:, bi, 0:16],
                  start=(bi == 0), stop=(bi == NB - 1))
    sc.activation(out=scr2[0:16, 0:16], in_=psSC[:], func=AF.Square,
                  bias=zb[0:16, :], accum_out=fro[0:16, 3:4])
    froR = sp.tile([32, 4], F32, tag="froR")
    gp.partition_all_reduce(froR[:], fro[:], channels=32, reduce_op=RED.add)
    gp.partition_all_reduce(LB16R[:], LB16[:], channels=16, reduce_op=RED.add)
    gp.partition_all_reduce(LBR2[:], LB[:], channels=128, reduce_op=RED.add)

    nums = sp.tile([1, 4], F32, tag="nums")
    dens = sp.tile([1, 4], F32, tag="dens")
    v.tensor_copy(nums[:, 0:1], LBR2[0:1, 0:1])
    v.tensor_tensor(out=nums[:, 1:2], in0=LBR2[0:1, 2:3], in1=LBR2[0:1, 3:4], op=ALU.subtract)
    v.tensor_copy(nums[:, 2:3], LB16R[0:1, 0:1])
    v.tensor_copy(nums[:, 3:4], LB16R[0:1, 1:2])
    v.tensor_copy(dens[:, 0:1], LBR2[0:1, 1:2])
    v.tensor_copy(dens[:, 1:2], LBR2[0:1, 4:5])
    v.tensor_copy(dens[:, 2:3], LBR2[0:1, 6:7])
    v.tensor_copy(dens[:, 3:4], LBR2[0:1, 10:11])
    v.tensor_scalar(out=dens[:], in0=dens[:], scalar1=float(EPS), scalar2=None, op0=ALU.add)
    drec = sp.tile([1, 4], F32, tag="drec")
    v.reciprocal(drec[:], dens[:])
    v.tensor_tensor(out=nums[:], in0=nums[:], in1=drec[:], op=ALU.mult)
    v.tensor_tensor(out=nums[:], in0=nums[:], in1=wt["signs4"][:], op=ALU.mult)
    mainv = sp.tile([1, 1], F32, tag="mainv")
    v.tensor_reduce(out=mainv[:], in_=_r1(nums), axis=AX.X, op=ALU.add)
    trs = sp.tile([1, 4], F32, tag="trs")
    v.tensor_copy(trs[:, 0:1], LBR2[0:1, 7:8])
    v.tensor_copy(trs[:, 1:2], LBR2[0:1, 4:5])
    v.tensor_copy(trs[:, 2:3], LBR2[0:1, 8:9])
    v.tensor_copy(trs[:, 3:4], LBR2[0:1, 9:10])
    nrm = sp.tile([1, 4], F32, tag="nrm")
    sc.activation(out=nrm[:], in_=froR[0:1, :], func=AF.Sqrt, bias=zb[0:1, :])
    nrec = sp.tile([1, 4], F32, tag="nrec")
    v.reciprocal(nrec[:], nrm[:])
    v.tensor_tensor(out=trs[:], in0=trs[:], in1=nrec[:], op=ALU.mult)
    v.tensor_tensor(out=trs[:], in0=trs[:], in1=wt["orthoc"][:], op=ALU.mult)
    v.tensor_scalar(out=trs[:], in0=trs[:], scalar1=2.0, scalar2=None, op0=ALU.add)
    orts = sp.tile([1, 4], F32, tag="orts")
    sc.activation(out=orts[:], in_=trs[:], func=AF.Sqrt, bias=zb[0:1, :])
    orthov = sp.tile([1, 1], F32, tag="orthov")
    v.tensor_reduce(out=orthov[:], in_=_r1(orts), axis=AX.X, op=ALU.add)
    lossrow = sp.tile([1, 2], F32, tag="lossrow")
    v.tensor_copy(lossrow[:, 0:1], mainv[:])
    v.tensor_copy(lossrow[:, 1:2], orthov[:])
    nc.sync.dma_start(out=out_loss[g:g + 1, :], in_=lossrow[:])
    if dbg is not None and g == 0:
        dbt = sp.tile([128, 2048], F32, tag="dbt")
        v.memset(dbt[:], 0.0)
        v.tensor_copy(dbt[:, 0:528], XEZ[:])                      # 0: XEZ
        v.tensor_copy(dbt[:, 528:544], srw[:])                    # 528: s_rw
        v.tensor_copy(dbt[:, 544:800], sct[:])                    # 544: s_ct
        v.tensor_copy(dbt[:, 800:932], AVW[:, 0:132])             # 800: AV block0
        v.tensor_copy(dbt[:, 932:964], CGW[:, 0:32])              # 932: conv_gap b0
        v.tensor_copy(dbt[:, 964:972], dG[:])                     # 964: dG
        v.tensor_copy(dbt[0:1, 972:983], LBR2[0:1, 0:11])         # 972: LB sums
        v.tensor_copy(dbt[0:33, 983:1111], convT[:, 0:128])       # 983: convT b0
        v.tensor_copy(dbt[:, 1111:1127], ZG[:, 0:16])             # 1111: z2g b0
        v.tensor_copy(dbt[:, 1127:1143], PQR[:, 0:16])            # 1127: s2g b0
        v.tensor_copy(dbt[:, 1143:1159], ZC[:, 0:16])             # 1143: z2c b0
        v.tensor_copy(dbt[:, 1159:1175], S2CB[:, 0:16])           # 1159: s2c b0
        v.tensor_copy(dbt[0:16, 1175:1191], oag[:])               # 1175: out_adj_g
        v.tensor_copy(dbt[0:16, 1191:1207], oac[:])               # 1191: out_adj_c
        v.tensor_copy(dbt[0:32, 1207:1223], t1T[:])               # t1T (last=ct)
        v.tensor_copy(dbt[0:1, 1223:1233], lrow[:])               # logits row
        v.tensor_copy(dbt[0:1, 1233:1237], nums[:])               # nums after mult
        v.tensor_copy(dbt[0:1, 1237:1241], dens[:])               # dens (+eps)
        v.tensor_copy(dbt[0:1, 1241:1245], froR[0:1, :])          # fro^2
        v.tensor_copy(dbt[0:16, 1245:1247], LB16R[:, 0:2])
        v.tensor_copy(dbt[:, 1261:1389], Vw3[:, 0, 0:128])
        v.tensor_copy(dbt[:, 1389:1406], CSn3[:, 0, :])
        nc.sync.dma_start(out=dbg[:, :], in_=dbt[:])


_NC_CACHE = None


def _get_nc():
    global _NC_CACHE
    if _NC_CACHE is None:
        _NC_CACHE = build_nc()
    return _NC_CACHE


def kernel(**inputs):
    from concourse.bass_utils import run_bass_kernel_spmd

    x = np.asarray(inputs["x"], np.float32)
    adj = np.asarray(inputs["adj"], np.float32)
    w = _fold_weights({k: v for k, v in inputs.items() if k not in ("x", "adj", "mask")})
    xt = np.ascontiguousarray(x.transpose(0, 2, 1))
    xt_h, xt_l = _hilo(xt)

    nc = _get_nc()
    in_maps = []
    for c in range(NCORES):
        s = slice(GPC * c, GPC * (c + 1))
        m = {"adj": np.ascontiguousarray(adj[s]),
             "xth": np.ascontiguousarray(xt_h[s]),
             "xtl": np.ascontiguousarray(xt_l[s])}
        for name, shape, dt in WSPECS:
            want = ml_dtypes.bfloat16 if dt == BF else np.float32
            m[name] = np.ascontiguousarray(np.asarray(w[name], dtype=want))
        in_maps.append(m)
    res = run_bass_kernel_spmd(nc, in_maps, core_ids=list(range(NCORES)))
    if DEBUG:
        kernel._dbg = res.results[0]["dbg"]
    ls = np.concatenate([res.results[c]["out_ls"] for c in range(NCORES)], axis=0)
    loss = np.concatenate([res.results[c]["out_loss"] for c in range(NCORES)], axis=0)
    return (ls.astype(np.float32), np.float32(loss[:, 0].mean()),
            np.float32(loss[:, 1].mean()))
